# revision 38
# baseline (speedup 1.0000x reference)
"""Multi-head self-attention on 8 Trainium2 NeuronCores.

Strategy (tensor parallel over heads, per the classic Megatron split):
  - 16 heads / 8 cores -> each core owns 2 heads (a 128-column slice of
    Wq/Wk/Wv and the matching 128-row slice of Wo).
  - x is transposed on the host to xT [D, B*S] in bf16 and replicated to
    every core; each core projects QT/KT/VT for its heads, runs attention
    for its (batch, head) pairs, and produces a partial output projection
    [B*S, D] in bf16.
  - Host sums the 8 partials in fp32 (the Wo row-parallel all-reduce) and
    adds bo.

Per-core kernel layout notes:
  - Everything is bf16 on the matmul operand side (fp32 PSUM accumulate):
    same 1 cycle/row PE stream rate as fp32r, but weight loads hit the
    fast-weight-load path (128-col bf16 stationaries) and DMA halves.
    Verified numerically: whole-pipeline bf16 is ~5e-3 rel rms vs fp32.
  - Scores are computed transposed, ST[k, q] = KT.T @ QT, two heads
    row-packed into the PE array (contraction is only 64 wide per head).
  - softmax denominator rides the attention matmul: the per-head
    stationary is [V_h | ones*64], so AV psum rows 64..127 all hold
    sum_k exp(s) -- a free partition-broadcast of the denominator that
    feeds the normalize directly (no gpsimd broadcast needed).
  - exp happens on ACT straight out of PSUM with the 1/8 logit scale,
    writing bf16.
"""
import sys

sys.path.insert(0, "/opt/trn_rl_repo")

import ml_dtypes
import numpy as np

import concourse.bacc as bacc
import concourse.tile as tile
from concourse import mybir
from concourse.bass_utils import run_bass_kernel_spmd
from concourse.masks import make_identity

AF = mybir.ActivationFunctionType
F32 = mybir.dt.float32
BF16 = mybir.dt.bfloat16
NP_BF16 = ml_dtypes.bfloat16

N_CORES = 8
EMBED_DIM = 1024
NUM_HEADS = 16
HEAD_DIM = 64


def build_attention_core(B, S, D, with_qkv_bias=False):
    """One core's program: 2 heads (E=128 projection slice) of MHA.

    B: batch, S: sequence length per batch, D: model dim.
    Inputs (bf16): xT [D, B*S], wq/wk/wv [128, D//128, 128], wo [128, D],
    bq/bk/bv [128].
    Output: out [B*S, D] bf16 (partial; host sums over cores in fp32).
    """
    P = 128          # partitions / d-chunk / k-tile
    E = 128          # per-core projection width (2 heads x 64)
    HD = 64          # head dim
    QC = 512         # q-chunk (matmul moving free dim)
    BS = B * S
    DC = D // P      # number of contraction chunks for projections
    n_sc = BS // QC  # s-chunks for projections
    n_kt = S // P    # k-tiles per batch
    n_qc = S // QC   # q-chunks per batch
    assert BS % QC == 0 and S % P == 0 and S % QC == 0 and D % P == 0
    scale = 1.0 / np.sqrt(np.float32(HD))

    nc = bacc.Bacc("TRN2", target_bir_lowering=False)
    xT = nc.dram_tensor("xT", [D, BS], BF16, kind="ExternalInput")
    # wq/wk/wv arrive pre-arranged on host as [P, DC, E] (partition-major)
    # so the load is one fully contiguous DMA instead of 512B descriptors.
    wq = nc.dram_tensor("wq", [P, D // P, E], BF16, kind="ExternalInput")
    wk = nc.dram_tensor("wk", [P, D // P, E], BF16, kind="ExternalInput")
    wv = nc.dram_tensor("wv", [P, D // P, E], BF16, kind="ExternalInput")
    wo = nc.dram_tensor("wo", [E, D], BF16, kind="ExternalInput")
    bq = nc.dram_tensor("bq", [E], BF16, kind="ExternalInput")
    bk = nc.dram_tensor("bk", [E], BF16, kind="ExternalInput")
    bv = nc.dram_tensor("bv", [E], BF16, kind="ExternalInput")
    out = nc.dram_tensor("out", [BS, D], BF16, kind="ExternalOutput")

    xT_r = xT[:].rearrange("(o p) s -> p o s", p=P)      # [P, DC, BS]
    w_r = {"q": wq[:], "k": wk[:], "v": wv[:]}

    with tile.TileContext(nc) as tc:
        with (
            tc.tile_pool(name="persist", bufs=1) as persist,
            tc.tile_pool(name="stage", bufs=2) as stage,
            tc.tile_pool(name="upool", bufs=5) as upool,
            tc.tile_pool(name="small", bufs=3) as small,
            tc.tile_pool(name="outp", bufs=3) as outp,
            tc.tile_pool(name="psA", bufs=2, space="PSUM") as psA,
            tc.tile_pool(name="psB", bufs=4, space="PSUM") as psB,
        ):
            # ---- x prefetch (first chunk queued before weight DMAs) ------
            _xpre = {}

            _xdone = set()

            def prefetch_x(sc):
                if sc in _xpre or sc in _xdone or sc >= n_sc:
                    return
                s0 = sc * QC
                xstage = stage.tile([P, DC, QC], BF16, tag="xstage")
                nsplit = min(4, DC)
                dper = DC // nsplit
                for sp in range(nsplit):
                    sl = slice(sp * dper, (sp + 1) * dper)
                    nc.sync.dma_start(xstage[:, sl, :], xT_r[:, sl, s0:s0 + QC])
                _xpre[sc] = xstage

            prefetch_x(0)

            # ---- constants & weights -------------------------------------
            # V-transpose runs in fp32 (PE transpose with 16-bit PSUM writes
            # diverges on HW even though CoreSim accepts it).
            ident = persist.tile([P, P], F32)
            make_identity(nc, ident[:])

            bias_t = {}
            if with_qkv_bias:
                for nm, b in (("q", bq), ("k", bk), ("v", bv)):
                    bt = persist.tile([P, 1], BF16, tag=f"bias_{nm}")
                    nc.sync.dma_start(bt[:], b[:].rearrange("(p o) -> p o", o=1))
                    bias_t[nm] = bt

            w_t = {}
            for nm in ("q", "k", "v"):
                wt = persist.tile([P, DC, E], BF16, tag=f"w_{nm}")
                nc.sync.dma_start(wt[:], w_r[nm])
                w_t[nm] = wt
            wo_t = persist.tile([E, D], BF16)
            nc.sync.dma_start(wo_t[:], wo[:])

            # ---- persistent activations ----------------------------------
            QT = persist.tile([P, BS], BF16, tag="QT")     # [e, s]
            KT = persist.tile([P, BS], BF16, tag="KT")     # [e, s]
            # Per-head AV stationary [ones*64 | V_h]: psum rows 0..63 of the
            # AV result replicate the softmax denominator sum_k exp(s), rows
            # 64..127 hold the unnormalized attention. Denominators sit at
            # partition base 0 because the custom-DVE reciprocal drops the
            # input AP's partition offset on hardware (CoreSim honors it).
            nch = BS // P
            Vaug = persist.tile([P, nch, 2, P], BF16, tag="Vaug")
            # contiguous memset; the V copies below overwrite cols HD:P, the
            # ones half stays.
            nc.gpsimd.memset(Vaug[:], 1.0)

            # ---- phase 1 projections (prefetch_x defined above) ------------
            def emit_proj_part(sc, nm):
                """Emit one of chunk sc's three projections (~2us of PE) so
                pieces can hide under the exp stream without starving ACT."""
                s0 = sc * QC
                if sc not in _xpre:
                    prefetch_x(sc)
                xtr = _xpre[sc]

                ps = psB.tile([P, QC], F32, tag="sc", bufs=2, name=f"ps_{nm}")
                for o in range(DC):
                    nc.tensor.matmul(
                        ps[:], w_t[nm][:, o, :], xtr[:, o, :],
                        start=(o == 0), stop=(o == DC - 1),
                    )

                def _bias_add(dst_ap):
                    if with_qkv_bias:
                        nc.vector.tensor_tensor(
                            dst_ap, dst_ap,
                            bias_t[nm][:, 0:1].to_broadcast((P, QC)),
                            mybir.AluOpType.add)

                if nm == "q":
                    nc.vector.tensor_copy(QT[:, s0:s0 + QC], ps[:])
                    _bias_add(QT[:, s0:s0 + QC])
                    # issue the next chunk's x load now (2-8 kt earlier than
                    # the v-part) — its arrival was measured just-in-time
                    # late at each chunk's first consumer. The previous
                    # chunk was popped at its v-part, so the stage ring
                    # still holds at most two chunks.
                    prefetch_x(sc + 1)
                elif nm == "k":
                    nc.vector.tensor_copy(KT[:, s0:s0 + QC], ps[:])
                    _bias_add(KT[:, s0:s0 + QC])
                else:
                    vt_sb = small.tile([P, QC], F32, tag="vt")
                    nc.vector.tensor_copy(vt_sb[:], ps[:])
                    _bias_add(vt_sb[:])
                    for ss in range(QC // P):
                        pt = psB.tile([P, P], F32, tag="sc", bufs=2, name="pt")
                        nc.tensor.transpose(
                            pt[:], vt_sb[:, ss * P:(ss + 1) * P], ident[:])
                        ch = sc * (QC // P) + ss
                        nc.vector.tensor_copy(
                            Vaug[:, ch, 0, HD:P], pt[:, 0:HD])
                        nc.vector.tensor_copy(
                            Vaug[:, ch, 1, HD:P], pt[:, HD:2 * HD])
                    _xpre.pop(sc)
                    _xdone.add(sc)
                    # keep the next chunk's x load ~4 kt ahead of its use
                    prefetch_x(sc + 1)

            def emit_proj(sc):
                for nm in ("q", "k", "v"):
                    emit_proj_part(sc, nm)

            # ---- phase 2: attention + output projection ------------------
            OCW = min(512, D)

            def emit_ktloop(b, qc, inject=None):
                """kt loop for one (batch, q-chunk) block. ``inject`` maps a
                kt index to callbacks (deferred outproj units of the previous
                block, or projection chunks) emitted mid-loop so their PE/DVE
                work hides under the ACT-bound exp stream."""
                q0 = b * S + qc * QC
                pa = [psB.tile([P, QC], F32, tag="pa", bufs=2, name=f"pa{h}")
                      for h in range(2)]

                def emit_av(kt, ut):
                    ch = (b * S) // P + kt
                    for h in range(2):
                        nc.tensor.matmul(
                            pa[h][:],
                            Vaug[:, ch, h, :],
                            ut[:, h * QC:(h + 1) * QC],
                            start=(kt == 0), stop=(kt == n_kt - 1))

                # AV is emitted two kt behind scores: PE streams ahead while
                # ACT's exp is in flight, and the first AV of a block (which
                # waits on the previous block's pa slot release) sits far
                # enough back in the PE queue not to stall the score stream.
                pending = []
                for kt in range(n_kt):
                    k0 = b * S + kt * P
                    st = psA.tile([P, 2 * QC], F32, tag="A", name="st")
                    nc.tensor.matmul(
                        st[:, 0:QC],
                        KT[0:HD, k0:k0 + P], QT[0:HD, q0:q0 + QC],
                        tile_position=(0, 0), start=True, stop=True)
                    nc.tensor.matmul(
                        st[:, QC:2 * QC],
                        KT[HD:2 * HD, k0:k0 + P], QT[HD:2 * HD, q0:q0 + QC],
                        tile_position=(64, 0), start=True, stop=True)
                    ut = upool.tile([P, 2 * QC], BF16, tag="U")
                    nc.scalar.activation(ut[:], st[:], AF.Exp, scale=float(scale))
                    pending.append((kt, ut))
                    if len(pending) > 2:
                        emit_av(*pending.pop(0))
                    if inject and kt in inject:
                        for fn in inject[kt]:
                            fn()
                for item in pending:
                    emit_av(*item)
                return pa

            def emit_normalize(b, qc, pa):
                # Frees pa: attn (rows HD:P) / denominator (rows 0:HD, at
                # partition base 0 for the custom-DVE reciprocal) -> attnT.
                attnT = small.tile([P, QC], BF16, tag="attnT")
                for h in range(2):
                    rinv = small.tile([HD, QC], F32, tag="rinv")
                    nc.vector.reciprocal_approx_fast(rinv[:], pa[h][0:HD, :])
                    nc.vector.tensor_tensor(
                        attnT[h * HD:(h + 1) * HD, :],
                        pa[h][HD:P, :], rinv[:],
                        mybir.AluOpType.mult)
                return attnT

            def make_outproj_units(b, qc, attnT):
                # One unit per 128-row output slab: 2 matmuls + casts + 1 DMA.
                q0 = b * S + qc * QC
                units = []
                for ss in range(QC // P):
                    def unit(ss=ss):
                        osb = outp.tile([P, D], BF16, tag="osb")
                        for oc in range(D // OCW):
                            po = psB.tile([P, OCW], F32, tag="sc", bufs=2,
                                          name="po")
                            nc.tensor.matmul(
                                po[:], attnT[:, ss * P:(ss + 1) * P],
                                wo_t[:, oc * OCW:(oc + 1) * OCW],
                                start=True, stop=True)
                            nc.vector.tensor_copy(
                                osb[:, oc * OCW:(oc + 1) * OCW], po[:])
                        nc.sync.dma_start(
                            out[q0 + ss * P:q0 + (ss + 1) * P, :], osb[:])
                    units.append(unit)
                return units

            # Schedule: block (b, qc) carries injected work so ACT never
            # starves. b0/qc0 interleaves the tail of batch-0's projections
            # with its own kt loop (scores of kt 4s..4s+3 only need chunk s,
            # so chunk c's parts must land before kt=4c); later blocks carry
            # the previous block's deferred outproj units and one of
            # batch-1's projection chunks, split into ~2us pieces matched to
            # ACT's exp-queue depth.
            # Projections are balanced against PE slack: chunks 0-1 run
            # upfront (deep exp queue from the first block), each batch's
            # qc0 block pipelines that batch's later chunks (scores of kt
            # 4s..4s+3 only need chunk s), and the chunks a batch needs
            # first are projected one to two blocks earlier.
            per_b = n_sc // B
            six = [1, 2, 3, 5, 6, 7]
            proj_parts = {(0, 0): [(sc, nm) for sc in range(1, per_b)
                                   for nm in ("q", "k", "v")]}
            part_kts = {(0, 0): [1, 2, 3, 5, 6, 7, 9, 10, 11]}
            if B > 1:
                for i, sc in enumerate(range(per_b, per_b + 2)):
                    blk = (0, n_qc - 2 + i)
                    proj_parts[blk] = [(sc, nm) for nm in ("q", "k", "v")]
                    part_kts[blk] = [1, 5, 9]
                proj_parts[(1, 0)] = [(sc, nm)
                                      for sc in range(per_b + 2, 2 * per_b)
                                      for nm in ("q", "k", "v")]
                part_kts[(1, 0)] = six
            prefetch_x(1)
            emit_proj(0)
            deferred = None
            for b in range(B):
                for qc in range(n_qc):
                    inject = {}
                    for kt, (sc, nm) in zip(part_kts.get((b, qc), []),
                                            proj_parts.get((b, qc), [])):
                        inject.setdefault(kt, []).append(
                            lambda sc=sc, nm=nm: emit_proj_part(sc, nm))
                    if deferred is not None:
                        ukts = ([4, 8, 12, 15] if len(inject) >= 6
                                else [2, 6, 10, 14])
                        for kt, u in zip(ukts, deferred):
                            inject.setdefault(kt, []).append(u)
                    pa = emit_ktloop(b, qc, inject)
                    attnT = emit_normalize(b, qc, pa)
                    deferred = make_outproj_units(b, qc, attnT)
            for u in deferred:
                u()

    nc.compile()
    return nc


_NC_CACHE = {}


def _get_nc(B, S, D, with_qkv_bias):
    key = (B, S, D, with_qkv_bias)
    if key not in _NC_CACHE:
        _NC_CACHE[key] = build_attention_core(B, S, D, with_qkv_bias)
    return _NC_CACHE[key]


def _pack_w(w):
    # [D, 128] -> [128, D//128, 128] partition-major for contiguous DMA
    D = w.shape[0]
    return np.ascontiguousarray(
        w.reshape(D // 128, 128, w.shape[1]).transpose(1, 0, 2)).astype(NP_BF16)


def run_attention(x, Wq, bq, Wk, bk, Wv, bv, Wo, bo, trace=False):
    B, S, D = x.shape
    with_qkv_bias = bool(np.any(bq) or np.any(bk) or np.any(bv))
    nc = _get_nc(B, S, D, with_qkv_bias)
    xT = np.ascontiguousarray(x.reshape(B * S, D).T).astype(NP_BF16)
    in_maps = []
    for c in range(N_CORES):
        sl = slice(c * 128, (c + 1) * 128)
        in_maps.append({
            "xT": xT,
            "wq": _pack_w(Wq[:, sl]),
            "wk": _pack_w(Wk[:, sl]),
            "wv": _pack_w(Wv[:, sl]),
            "wo": np.ascontiguousarray(Wo[sl, :]).astype(NP_BF16),
            "bq": np.ascontiguousarray(bq[sl]).astype(NP_BF16),
            "bk": np.ascontiguousarray(bk[sl]).astype(NP_BF16),
            "bv": np.ascontiguousarray(bv[sl]).astype(NP_BF16),
        })
    res = run_bass_kernel_spmd(nc, in_maps, core_ids=list(range(N_CORES)),
                               trace=trace)
    acc = np.asarray(res.results[0]["out"]).astype(np.float32)
    for c in range(1, N_CORES):
        acc = acc + np.asarray(res.results[c]["out"]).astype(np.float32)
    acc = acc + np.asarray(bo, dtype=np.float32)[None, :]
    return acc.reshape(B, S, D), res


def kernel(x, Wq, bq, Wk, bk, Wv, bv, Wo, bo):
    out, _ = run_attention(np.asarray(x), np.asarray(Wq), np.asarray(bq),
                           np.asarray(Wk), np.asarray(bk), np.asarray(Wv),
                           np.asarray(bv), np.asarray(Wo), np.asarray(bo))
    return out
